# revision 7
# baseline (speedup 1.0000x reference)
"""Trainium2 Bass kernel for LowRankBilinearAttention.

Math (reference):
    p1 = x1 @ W1.T                  (B, P, A)
    p2 = x2 @ W2.T                  (B, L, A)
    m  = tanh(p1[:,None,:,:] * p2[:,:,None,:])      (B, L, P, A)
    h  = m @ Wh.T + bh
    t  = h @ Wt[0] + bt[0]          (B, L, P)
    alpha = softmax(t / TAU, axis=-1)
    label = alpha @ x1              (B, L, D1)
    return (label, alpha)

Key algebraic reduction (exact): t = m @ (Wt[0] @ Wh) + const, and the
constant shift cancels in the softmax, so with v = Wt[0] @ Wh (A,):
    t[b,l,p] ~ sum_a v[a] * tanh(p1[b,p,a] * p2[b,l,a])
which removes the (B,L,P,A)x(A,A) matmul entirely.

Sharding: pure data parallel, B=16 split as 2 batches per core across 8 cores.

Device layout per core (BL=2 batches):
  - p1T (a on partitions, p free) and p2T (a, l) via TensorE matmuls.
  - hot loop over (ac, l): u = p1T * p2T[:, l]  (VectorE tensor_scalar, fp32 2x)
    tanh in-place on big tiles (ScalarE), then per-l masked-stationary matmul
    (32-column sliding window over a v-mask) accumulating t (80+16pad, 196)
    in PSUM across the 4 a-chunks.
  - softmax: reduce_max (negated) -> exp with bias & fused accumulated sum ->
    reciprocal -> tensor_scalar mul.
  - label: TensorE transpose of alpha, then alphaT.T @ x1 (natural layout).
"""

import numpy as np

import concourse.bass as bass
import concourse.bacc as bacc
import concourse.mybir as mybir
import concourse.tile as tile
from concourse import bass_utils

F32 = mybir.dt.float32

B = 16
P = 196
L = 80
D1 = 2048
D2 = 300
A = 512
NCORES = 8
BL = B // NCORES  # batches per core
AC = A // 128     # a-chunks
DC = D1 // 128    # d-chunks for x1/W1
LG = 16           # l-group size for the hot loop
NG = L // LG      # number of l groups


def _build_device_program(nc):
    x1 = nc.dram_tensor("x1", (BL, P, D1), F32, kind="ExternalInput")
    x1t = nc.dram_tensor("x1t", (BL, D1, P), F32, kind="ExternalInput")
    x2t = nc.dram_tensor("x2t", (BL, D2, L), F32, kind="ExternalInput")
    w1t = nc.dram_tensor("w1t", (D1, A), F32, kind="ExternalInput")
    w2t = nc.dram_tensor("w2t", (D2, A), F32, kind="ExternalInput")
    zmask = nc.dram_tensor("zmask", (128, AC, 176), F32, kind="ExternalInput")
    ident = nc.dram_tensor("ident", (128, 128), F32, kind="ExternalInput")
    label_o = nc.dram_tensor("label", (BL, L, D1), F32, kind="ExternalOutput")
    alpha_o = nc.dram_tensor("alpha", (BL, L, P), F32, kind="ExternalOutput")

    with tile.TileContext(nc) as tc:
        with (
            tc.tile_pool(name="consts", bufs=1) as consts,
            tc.tile_pool(name="x1t_pool", bufs=2) as x1t_pool,
            tc.tile_pool(name="x2t_pool", bufs=2) as x2t_pool,
            tc.tile_pool(name="x1_pool", bufs=2) as x1_pool,
            tc.tile_pool(name="p1_pool", bufs=2) as p1_pool,
            tc.tile_pool(name="p2_pool", bufs=2) as p2_pool,
            tc.tile_pool(name="u_pool", bufs=3) as u_pool,
            tc.tile_pool(name="sm_pool", bufs=4) as sm_pool,
            tc.tile_pool(name="alpha_pool", bufs=2) as alpha_pool,
            tc.tile_pool(name="at_pool", bufs=2) as at_pool,
            tc.tile_pool(name="lab_pool", bufs=2) as lab_pool,
            tc.tile_pool(name="ps_a", bufs=2, space="PSUM") as ps_a,
            tc.tile_pool(name="ps_t", bufs=2, space="PSUM") as ps_t,
            tc.tile_pool(name="ps_m", bufs=2, space="PSUM") as ps_m,
        ):
            # ---- constants ----
            w1t_sb = consts.tile([128, DC, A], F32)
            nc.sync.dma_start(
                out=w1t_sb, in_=w1t.ap().rearrange("(dc p) a -> p dc a", p=128)
            )
            w2t_sb = consts.tile([128, 3, A], F32)
            nc.sync.dma_start(out=w2t_sb[0:128, 0, :], in_=w2t.ap()[0:128, :])
            nc.sync.dma_start(out=w2t_sb[0:128, 1, :], in_=w2t.ap()[128:256, :])
            nc.sync.dma_start(out=w2t_sb[0:44, 2, :], in_=w2t.ap()[256:300, :])
            zmask_sb = consts.tile([128, AC, 176], F32)
            nc.sync.dma_start(out=zmask_sb, in_=zmask.ap())
            ident_sb = consts.tile([128, 128], F32)
            nc.sync.dma_start(out=ident_sb, in_=ident.ap())

            for b in range(BL):
                # ---- input loads ----
                x1t_sb = x1t_pool.tile([128, DC, P], F32, tag="x1t_sb")
                nc.sync.dma_start(
                    out=x1t_sb,
                    in_=x1t.ap()[b].rearrange("(dc p) q -> p dc q", p=128),
                )
                x2t_sb = x2t_pool.tile([128, 3, L], F32, tag="x2t_sb")
                nc.sync.dma_start(out=x2t_sb[0:128, 0, :], in_=x2t.ap()[b, 0:128, :])
                nc.sync.dma_start(out=x2t_sb[0:128, 1, :], in_=x2t.ap()[b, 128:256, :])
                nc.sync.dma_start(out=x2t_sb[0:44, 2, :], in_=x2t.ap()[b, 256:300, :])
                x1_sb = x1_pool.tile([128, 2, D1], F32, tag="x1_sb")
                nc.sync.dma_start(out=x1_sb[:, 0, :], in_=x1.ap()[b, 0:128, :])
                nc.sync.dma_start(out=x1_sb[0:68, 1, :], in_=x1.ap()[b, 128:196, :])

                # ---- p1T (a on partitions, p on free), per a-chunk ----
                p1_sb = p1_pool.tile([128, AC, P], F32, tag="p1_sb")
                for ac in range(AC):
                    ps = ps_a.tile([128, P], F32, tag="ps")
                    for dc in range(DC):
                        nc.tensor.matmul(
                            ps,
                            lhsT=w1t_sb[:, dc, ac * 128 : (ac + 1) * 128],
                            rhs=x1t_sb[:, dc, :],
                            start=(dc == 0),
                            stop=(dc == DC - 1),
                        )
                    nc.vector.tensor_copy(p1_sb[:, ac, :], ps)

                # ---- p2T (a on partitions, l on free), per a-chunk ----
                p2_sb = p2_pool.tile([128, AC, L], F32, tag="p2_sb")
                for ac in range(AC):
                    ps = ps_a.tile([128, L], F32, tag="ps")
                    a_sl = slice(ac * 128, (ac + 1) * 128)
                    nc.tensor.matmul(
                        ps, lhsT=w2t_sb[0:128, 0, a_sl], rhs=x2t_sb[0:128, 0, :],
                        start=True, stop=False,
                    )
                    nc.tensor.matmul(
                        ps, lhsT=w2t_sb[0:128, 1, a_sl], rhs=x2t_sb[0:128, 1, :],
                        start=False, stop=False,
                    )
                    nc.tensor.matmul(
                        ps, lhsT=w2t_sb[0:44, 2, a_sl], rhs=x2t_sb[0:44, 2, :],
                        start=False, stop=True,
                    )
                    nc.vector.tensor_copy(p2_sb[:, ac, :], ps)

                # ---- hot loop: t[l,p] = sum_a v[a] tanh(p1T[a,p] p2T[a,l]) ----
                # t accumulates in PSUM; row-group base must be a multiple of 32,
                # so l lands at partition l via a 32/16-wide masked stationary.
                t_ps = ps_t.tile([96, P], F32, tag="t_ps")
                for ac in range(AC):
                    for g in range(NG):
                        u = u_pool.tile([128, LG, P], F32, tag="u")
                        for j in range(LG):
                            l = g * LG + j
                            nc.vector.tensor_scalar_mul(
                                u[:, j, :],
                                p1_sb[:, ac, :],
                                p2_sb[:, ac, l : l + 1],
                            )
                        nc.scalar.activation(
                            u, u, mybir.ActivationFunctionType.Tanh
                        )
                        for j in range(LG):
                            l = g * LG + j
                            # Each matmul writes all 96 rows: the stationary
                            # is a 96-col window of the v-mask, nonzero only
                            # at column l, so row l gets v . u and the other
                            # rows accumulate zeros. One accumulation group
                            # per (batch) bank across all 320 matmuls.
                            nc.tensor.matmul(
                                t_ps[0:96, :],
                                lhsT=zmask_sb[:, ac, 80 - l : 80 - l + 96],
                                rhs=u[:, j, :],
                                start=(ac == 0 and l == 0),
                                stop=(ac == AC - 1 and l == L - 1),
                            )

                # ---- softmax over p ----
                negmax = sm_pool.tile([L, 1], F32, tag="sm1")
                nc.vector.tensor_reduce(
                    negmax, t_ps[0:L, :], axis=mybir.AxisListType.X,
                    op=mybir.AluOpType.max, negate=True,
                )
                e_sb = sm_pool.tile([L, P], F32, tag="sme")
                ssum = sm_pool.tile([L, 1], F32, tag="sm2")
                nc.scalar.activation(
                    e_sb, t_ps[0:L, :], mybir.ActivationFunctionType.Exp,
                    bias=negmax, scale=1.0, accum_out=ssum,
                )
                rs = sm_pool.tile([L, 1], F32, tag="sm3")
                nc.vector.reciprocal(rs, ssum)
                alpha_sb = alpha_pool.tile([L, P], F32, tag="alpha_sb")
                nc.vector.tensor_scalar_mul(alpha_sb, e_sb, rs)
                nc.sync.dma_start(out=alpha_o.ap()[b], in_=alpha_sb)

                # ---- label = alpha @ x1 : lhsT = alphaT (p on partitions) ----
                at_sb = at_pool.tile([128, 2, L], F32, tag="at_sb")
                aps0 = ps_m.tile([128, L], F32, tag="psm")
                nc.tensor.transpose(aps0, alpha_sb[:, 0:128], ident_sb[0:L, 0:L])
                nc.vector.tensor_copy(at_sb[:, 0, :], aps0)
                aps1 = ps_m.tile([68, L], F32, tag="psm")
                nc.tensor.transpose(aps1, alpha_sb[:, 128:196], ident_sb[0:L, 0:L])
                nc.vector.tensor_copy(at_sb[0:68, 1, :], aps1)

                label_sb = lab_pool.tile([L, D1], F32, tag="label_sb")
                for n4 in range(4):
                    lp = ps_m.tile([L, 512], F32, tag="psm")
                    d_sl = slice(n4 * 512, (n4 + 1) * 512)
                    nc.tensor.matmul(
                        lp, lhsT=at_sb[:, 0, :], rhs=x1_sb[:, 0, d_sl],
                        start=True, stop=False,
                    )
                    nc.tensor.matmul(
                        lp, lhsT=at_sb[0:68, 1, :], rhs=x1_sb[0:68, 1, d_sl],
                        start=False, stop=True,
                    )
                    nc.vector.tensor_copy(label_sb[:, d_sl], lp)
                nc.sync.dma_start(out=label_o.ap()[b], in_=label_sb)

    return nc


_NC_CACHE = None


def build_nc():
    global _NC_CACHE
    if _NC_CACHE is None:
        nc = bacc.Bacc(
            "TRN2",
            target_bir_lowering=False,
            debug=False,
            enable_asserts=False,
            num_devices=NCORES,
        )
        _build_device_program(nc)
        nc.compile()
        _NC_CACHE = nc
    return _NC_CACHE


def make_in_maps(x1, x2, W1, W2, Wh, bh, Wt, bt):
    x1 = np.asarray(x1, dtype=np.float32)
    x2 = np.asarray(x2, dtype=np.float32)
    W1 = np.asarray(W1, dtype=np.float32)
    W2 = np.asarray(W2, dtype=np.float32)
    Wh = np.asarray(Wh, dtype=np.float32)
    Wt = np.asarray(Wt, dtype=np.float32)

    v = (Wt[0] @ Wh).astype(np.float32)  # (A,) ; bias terms cancel in softmax
    zmask = np.zeros((128, AC, 176), dtype=np.float32)
    for c in range(AC):
        zmask[:, c, 80] = v[c * 128 : (c + 1) * 128]
    ident = np.eye(128, dtype=np.float32)
    w1t = np.ascontiguousarray(W1.T)
    w2t = np.ascontiguousarray(W2.T)

    in_maps = []
    for k in range(NCORES):
        sl = slice(k * BL, (k + 1) * BL)
        in_maps.append(
            {
                "x1": np.ascontiguousarray(x1[sl]),
                "x1t": np.ascontiguousarray(x1[sl].transpose(0, 2, 1)),
                "x2t": np.ascontiguousarray(x2[sl].transpose(0, 2, 1)),
                "w1t": w1t,
                "w2t": w2t,
                "zmask": zmask,
                "ident": ident,
            }
        )
    return in_maps


def run(inputs, trace=False, **kw):
    nc = build_nc()
    in_maps = make_in_maps(**inputs)
    res = bass_utils.run_bass_kernel_spmd(
        nc, in_maps, core_ids=list(range(NCORES)), trace=trace, **kw
    )
    label = np.concatenate([r["label"] for r in res.results], axis=0)
    alpha = np.concatenate([r["alpha"] for r in res.results], axis=0)
    return (label, alpha), res


def kernel(**inputs):
    out, _ = run(inputs, trace=False)
    return out


# revision 14
# speedup vs baseline: 1.5755x; 1.5755x over previous
"""Trainium2 Bass kernel for LowRankBilinearAttention.

Math (reference):
    p1 = x1 @ W1.T                  (B, P, A)
    p2 = x2 @ W2.T                  (B, L, A)
    m  = tanh(p1[:,None,:,:] * p2[:,:,None,:])      (B, L, P, A)
    h  = m @ Wh.T + bh
    t  = h @ Wt[0] + bt[0]          (B, L, P)
    alpha = softmax(t / TAU, axis=-1)
    label = alpha @ x1              (B, L, D1)
    return (label, alpha)

Key algebraic reduction (exact): t = m @ (Wt[0] @ Wh) + const, and the
constant shift cancels in the softmax, so with v = Wt[0] @ Wh (A,):
    t[b,l,p] ~ sum_a v[a] * tanh(p1[b,p,a] * p2[b,l,a])
which removes the (B,L,P,A)x(A,A) matmul entirely.

Sharding: pure data parallel, B=16 split as 2 batches per core across 8 cores.

Device schedule per core (BL=2 batches, paired into one 392-col moving
operand so float32r matmuls hit the fast 1-cycle/row path):
  - p1T (a on partitions, (b,p) free) and p2T (a, (b,l)) via TensorE matmuls.
  - hot loop per (a-chunk, l-group): one big stride-0-broadcast tensor_tensor
    multiply u[a, l, b, p] = p1T[a,b,p] * p2T[a,b,l] (VectorE), tanh in-place
    (ScalarE), then per-l matmul with a 96-row masked sliding v-window
    stationary (float32r) accumulating t (96, 392) in PSUM over a-chunks.
  - per batch: softmax (reduce_max negated -> exp with bias + fused sum ->
    reciprocal -> scalar mul), TensorE transpose of alpha, label matmul.
"""

import numpy as np

import concourse.bass as bass
import concourse.bacc as bacc
import concourse.mybir as mybir
import concourse.tile as tile
from concourse import bass_utils

F32 = mybir.dt.float32
F32R = mybir.dt.float32r

B = 16
P = 196
L = 80
D1 = 2048
D2 = 300
A = 512
NCORES = 8
BL = B // NCORES  # batches per core
AC = A // 128     # a-chunks
DC = D1 // 128    # d-chunks for x1/W1
LG = 8            # l-group size for the hot loop
NG = L // LG      # number of l groups
BP = BL * P       # paired free width (392)


def _bcast(base, free_dims, extra_offset=0):
    """AP with base's partition dim and explicit free [step, count] dims
    (step 0 = broadcast)."""
    part = list(base.ap[0])
    return bass.AP(
        tensor=base.tensor,
        offset=base.offset + extra_offset,
        ap=[part] + [list(d) for d in free_dims],
    )


def _build_device_program(nc):
    x1 = nc.dram_tensor("x1", (BL, P, D1), F32R, kind="ExternalInput")
    x1t = nc.dram_tensor("x1t", (BL, D1, P), F32R, kind="ExternalInput")
    x2t = nc.dram_tensor("x2t", (BL, D2, L), F32, kind="ExternalInput")
    w1t = nc.dram_tensor("w1t", (D1, A), F32R, kind="ExternalInput")
    w2t = nc.dram_tensor("w2t", (D2, A), F32, kind="ExternalInput")
    zmask = nc.dram_tensor("zmask", (128, AC, 176), F32R, kind="ExternalInput")
    ident = nc.dram_tensor("ident", (128, 128), F32, kind="ExternalInput")
    label_o = nc.dram_tensor("label", (BL, L, D1), F32, kind="ExternalOutput")
    alpha_o = nc.dram_tensor("alpha", (BL, L, P), F32, kind="ExternalOutput")

    with tile.TileContext(nc) as tc:
        with (
            tc.tile_pool(name="consts", bufs=1) as consts,
            tc.tile_pool(name="x1_pool", bufs=2) as x1_pool,
            tc.tile_pool(name="u_pool", bufs=2) as u_pool,
            tc.tile_pool(name="m_pool", bufs=2) as m_pool,
            tc.tile_pool(name="sm_pool", bufs=4) as sm_pool,
            tc.tile_pool(name="alpha_pool", bufs=2) as alpha_pool,
            tc.tile_pool(name="at_pool", bufs=2) as at_pool,
            tc.tile_pool(name="lab_pool", bufs=2) as lab_pool,
            tc.tile_pool(name="ps_a", bufs=2, space="PSUM") as ps_a,
            tc.tile_pool(name="ps_t", bufs=2, space="PSUM") as ps_t,
            tc.tile_pool(name="ps_m", bufs=2, space="PSUM") as ps_m,
        ):
            # ---- constants ----
            w1t_sb = consts.tile([128, DC, A], F32R)
            nc.sync.dma_start(
                out=w1t_sb, in_=w1t.ap().rearrange("(dc p) a -> p dc a", p=128)
            )
            w2t_sb = consts.tile([128, 3, A], F32)
            nc.sync.dma_start(out=w2t_sb[0:128, 0, :], in_=w2t.ap()[0:128, :])
            nc.sync.dma_start(out=w2t_sb[0:128, 1, :], in_=w2t.ap()[128:256, :])
            nc.sync.dma_start(out=w2t_sb[0:44, 2, :], in_=w2t.ap()[256:300, :])
            zmask_sb = consts.tile([128, AC, 176], F32R)
            nc.sync.dma_start(out=zmask_sb, in_=zmask.ap())
            ident_sb = consts.tile([128, 128], F32)
            nc.sync.dma_start(out=ident_sb, in_=ident.ap())

            # ---- paired input loads (both batches in one tile) ----
            x1t_sb = consts.tile([128, DC, BL, P], F32R)
            for b in range(BL):
                nc.sync.dma_start(
                    out=x1t_sb[:, :, b, :],
                    in_=x1t.ap()[b].rearrange("(dc p) q -> p dc q", p=128),
                )
            x2t_sb = consts.tile([128, 3, BL, L], F32)
            for b in range(BL):
                nc.sync.dma_start(
                    out=x2t_sb[0:128, 0, b, :], in_=x2t.ap()[b, 0:128, :]
                )
                nc.sync.dma_start(
                    out=x2t_sb[0:128, 1, b, :], in_=x2t.ap()[b, 128:256, :]
                )
                nc.sync.dma_start(
                    out=x2t_sb[0:44, 2, b, :], in_=x2t.ap()[b, 256:300, :]
                )
            x1_sbs = []
            for b in range(BL):
                x1_sb = x1_pool.tile([128, 2, D1], F32R, tag="x1_sb")
                nc.sync.dma_start(out=x1_sb[:, 0, :], in_=x1.ap()[b, 0:128, :])
                nc.sync.dma_start(out=x1_sb[0:68, 1, :], in_=x1.ap()[b, 128:196, :])
                x1_sbs.append(x1_sb)

            # ---- p1T (a partitions, (b,p) free), float32r fast path ----
            p1_sb = consts.tile([128, AC, BL, P], F32)
            for ac in range(AC):
                ps = ps_a.tile([128, BP], F32, tag="ps")
                for dc in range(DC):
                    nc.tensor.matmul(
                        ps,
                        lhsT=w1t_sb[:, dc, ac * 128 : (ac + 1) * 128],
                        rhs=_bcast(x1t_sb[:, dc, :, :], [[1, BP]]),
                        start=(dc == 0),
                        stop=(dc == DC - 1),
                    )
                nc.vector.tensor_copy(p1_sb[:, ac, :, :], ps)

            # ---- p2T (a partitions, (b,l) free) ----
            p2_sb = consts.tile([128, AC, BL, L], F32)
            for ac in range(AC):
                ps = ps_a.tile([128, BL * L], F32, tag="ps")
                a_sl = slice(ac * 128, (ac + 1) * 128)
                nc.tensor.matmul(
                    ps,
                    lhsT=w2t_sb[0:128, 0, a_sl],
                    rhs=_bcast(x2t_sb[0:128, 0, :, :], [[1, BL * L]]),
                    start=True, stop=False,
                )
                nc.tensor.matmul(
                    ps,
                    lhsT=w2t_sb[0:128, 1, a_sl],
                    rhs=_bcast(x2t_sb[0:128, 1, :, :], [[1, BL * L]]),
                    start=False, stop=False,
                )
                nc.tensor.matmul(
                    ps,
                    lhsT=w2t_sb[0:44, 2, a_sl],
                    rhs=_bcast(x2t_sb[0:44, 2, :, :], [[1, BL * L]]),
                    start=False, stop=True,
                )
                nc.vector.tensor_copy(p2_sb[:, ac, :, :], ps)

            # ---- hot loop: t[l, (b,p)] += v_ac . tanh(p1T * p2T) ----
            t_ps = ps_t.tile([96, BP], F32, tag="t_ps")
            for ac in range(AC):
                for g in range(NG):
                    u = u_pool.tile([128, LG, BL, P], F32, tag="u")
                    # broadcast p1 over the l-group (step-0 dim), p2 over p
                    in0 = _bcast(p1_sb[:, ac, :, :], [[0, LG], [P, BL], [1, P]])
                    in1 = _bcast(
                        p2_sb[:, ac, :, :],
                        [[1, LG], [L, BL], [0, P]],
                        extra_offset=g * LG,
                    )
                    nc.vector.tensor_tensor(
                        out=u[:, :, :, :], in0=in0, in1=in1,
                        op=mybir.AluOpType.mult,
                    )
                    m = m_pool.tile([128, LG, BL, P], F32R, tag="m")
                    nc.scalar.activation(
                        m, u, mybir.ActivationFunctionType.Tanh
                    )
                    for j in range(LG):
                        l = g * LG + j
                        nc.tensor.matmul(
                            t_ps[0:96, :],
                            lhsT=zmask_sb[:, ac, 80 - l : 80 - l + 96],
                            rhs=m[:, j, :, :],
                            start=(ac == 0 and l == 0),
                            stop=(ac == AC - 1 and l == L - 1),
                        )

            # ---- per batch: softmax + label ----
            for b in range(BL):
                t_sl = t_ps[0:L, b * P : (b + 1) * P]
                negmax = sm_pool.tile([L, 1], F32, tag="sm1")
                nc.vector.tensor_reduce(
                    negmax, t_sl, axis=mybir.AxisListType.X,
                    op=mybir.AluOpType.max, negate=True,
                )
                e_sb = sm_pool.tile([L, P], F32, tag="sme")
                ssum = sm_pool.tile([L, 1], F32, tag="sm2")
                nc.scalar.activation(
                    e_sb, t_sl, mybir.ActivationFunctionType.Exp,
                    bias=negmax, scale=1.0, accum_out=ssum,
                )
                rs = sm_pool.tile([L, 1], F32, tag="sm3")
                nc.vector.reciprocal(rs, ssum)
                alpha_sb = alpha_pool.tile([L, P], F32, tag="alpha_sb")
                nc.vector.tensor_scalar_mul(alpha_sb, e_sb, rs)
                nc.sync.dma_start(out=alpha_o.ap()[b], in_=alpha_sb)

                # label = alpha @ x1 : lhsT = alphaT (p on partitions)
                at_sb = at_pool.tile([128, 2, L], F32R, tag="at_sb")
                aps0 = ps_m.tile([128, L], F32, tag="psm")
                nc.tensor.transpose(aps0, alpha_sb[:, 0:128], ident_sb[0:L, 0:L])
                nc.vector.tensor_copy(at_sb[:, 0, :], aps0)
                aps1 = ps_m.tile([68, L], F32, tag="psm")
                nc.tensor.transpose(aps1, alpha_sb[:, 128:196], ident_sb[0:L, 0:L])
                nc.vector.tensor_copy(at_sb[0:68, 1, :], aps1)

                label_sb = lab_pool.tile([L, D1], F32, tag="label_sb")
                for n4 in range(4):
                    lp = ps_m.tile([L, 512], F32, tag="psm")
                    d_sl = slice(n4 * 512, (n4 + 1) * 512)
                    nc.tensor.matmul(
                        lp,
                        lhsT=at_sb[:, 0, :],
                        rhs=x1_sbs[b][:, 0, d_sl],
                        start=True, stop=False,
                    )
                    nc.tensor.matmul(
                        lp,
                        lhsT=at_sb[0:68, 1, :],
                        rhs=x1_sbs[b][0:68, 1, d_sl],
                        start=False, stop=True,
                    )
                    nc.vector.tensor_copy(label_sb[:, d_sl], lp)
                nc.sync.dma_start(out=label_o.ap()[b], in_=label_sb)

    return nc


_NC_CACHE = None


def build_nc():
    global _NC_CACHE
    if _NC_CACHE is None:
        nc = bacc.Bacc(
            "TRN2",
            target_bir_lowering=False,
            debug=False,
            enable_asserts=False,
            num_devices=NCORES,
        )
        _build_device_program(nc)
        nc.compile()
        _NC_CACHE = nc
    return _NC_CACHE


def make_in_maps(x1, x2, W1, W2, Wh, bh, Wt, bt):
    x1 = np.asarray(x1, dtype=np.float32)
    x2 = np.asarray(x2, dtype=np.float32)
    W1 = np.asarray(W1, dtype=np.float32)
    W2 = np.asarray(W2, dtype=np.float32)
    Wh = np.asarray(Wh, dtype=np.float32)
    Wt = np.asarray(Wt, dtype=np.float32)

    v = (Wt[0] @ Wh).astype(np.float32)  # (A,) ; bias terms cancel in softmax
    zmask = np.zeros((128, AC, 176), dtype=np.float32)
    for c in range(AC):
        zmask[:, c, 80] = v[c * 128 : (c + 1) * 128]
    ident = np.eye(128, dtype=np.float32)
    w1t = np.ascontiguousarray(W1.T)
    w2t = np.ascontiguousarray(W2.T)

    in_maps = []
    for k in range(NCORES):
        sl = slice(k * BL, (k + 1) * BL)
        in_maps.append(
            {
                "x1": np.ascontiguousarray(x1[sl]),
                "x1t": np.ascontiguousarray(x1[sl].transpose(0, 2, 1)),
                "x2t": np.ascontiguousarray(x2[sl].transpose(0, 2, 1)),
                "w1t": w1t,
                "w2t": w2t,
                "zmask": zmask,
                "ident": ident,
            }
        )
    return in_maps


def run(inputs, trace=False, **kw):
    nc = build_nc()
    in_maps = make_in_maps(**inputs)
    res = bass_utils.run_bass_kernel_spmd(
        nc, in_maps, core_ids=list(range(NCORES)), trace=trace, **kw
    )
    label = np.concatenate([r["label"] for r in res.results], axis=0)
    alpha = np.concatenate([r["alpha"] for r in res.results], axis=0)
    return (label, alpha), res


def kernel(**inputs):
    out, _ = run(inputs, trace=False)
    return out
